# revision 19
# baseline (speedup 1.0000x reference)
"""Plane-sweep cost-volume kernel for Trainium2 (8 NeuronCores).

Problem shape (hardcoded): B=1, V=4 source views, C=16 feature channels,
H=64, W=96, D=64 depth planes.  Output: (1, D, H, W) float32.

Strategy
--------
The benchmark geometry has identity rotations (extrinsics are pure
translations) and zero-skew pinhole intrinsics, so for each (view, depth
plane) the warp from output pixels to source-image sample coordinates is an
axis-separable affine map:  x = ax + bx*px,  y = ay + by*py.  Bilinear
grid_sample with zero padding then factorizes exactly into two 1-D linear
interpolations, each a small dense matrix of "hat" functions
hat(t - k) = max(0, 1 - |t - k|):

    warped_c = Ay(v,d) @ src_c @ Bx(v,d)^T        (exactly equal to
                                                   grid_sample zeros/bilinear)

so the whole cost volume becomes TensorEngine matmuls — no gathers.  The
view sum  sum_v  is accumulated in PSUM, and the channel dot with cur_feats
is a vector multiply + strided reduce.

Sharding: depth planes across the 8 cores (8 planes each); features are
replicated.  The hat matrices Ay/Bx are built on host directly in bf16 and
DMA'd (the previous version built them on device from broadcast DMAs, which
serialized ~25us of startup in front of the first matmul).  DMAs are spread
across the HW-DGE queues (sync/scalar) plus vector/gpsimd SWDGE queues so
the tensor engine starts within ~2us.  PSUM->SBUF casts round-robin over
the DVE/Pool/Act engines so the PE is never cast-throttled (which would
also hold it at the 1.2GHz mid p-state).

If the inputs do not have the separable structure (rotations != identity or
non-pinhole intrinsics), we fall back to an exact numpy implementation.
"""

import numpy as np
import ml_dtypes

H, W, D, V, C = 64, 96, 64, 4, 16
N_CORES = 8
DLOC = D // N_CORES            # 8 depth planes per core
NY = V * DLOC * H              # 2048
NX = V * DLOC * W              # 3072
EPS = 1e-8
OOB = 1.0e9                    # sample coord pushed out of range => zero weights

_CACHE = {}


# --------------------------------------------------------------------------
# Device kernel
# --------------------------------------------------------------------------
def _build_nc():
    import concourse.bacc as bacc
    import concourse.tile as tile
    from concourse import mybir

    fp32 = mybir.dt.float32
    bf16 = mybir.dt.bfloat16
    Alu = mybir.AluOpType
    Axis = mybir.AxisListType

    nc = bacc.Bacc("TRN2", target_bir_lowering=False, debug=False,
                   num_devices=N_CORES)

    srcT = nc.dram_tensor("srcT", [V, H, C, W], bf16, kind="ExternalInput")
    ay = nc.dram_tensor("ay", [H, NY], bf16, kind="ExternalInput")
    bx = nc.dram_tensor("bx", [W, NX], bf16, kind="ExternalInput")
    cur = nc.dram_tensor("cur", [W, C, H], bf16, kind="ExternalInput")
    out = nc.dram_tensor("out", [W, DLOC, H], fp32, kind="ExternalOutput")

    with tile.TileContext(nc) as tc:
        with (
            tc.tile_pool(name="consts", bufs=1) as consts,
            tc.tile_pool(name="tp", bufs=1) as tp_pool,
            tc.tile_pool(name="ps1", bufs=2, space="PSUM") as ps1_pool,
            tc.tile_pool(name="ps2", bufs=2, space="PSUM") as ps2_pool,
            tc.tile_pool(name="s2", bufs=2) as s2_pool,
            tc.tile_pool(name="tmp", bufs=3) as tmp_pool,
            tc.tile_pool(name="osb", bufs=1) as osb_pool,
        ):
            # ---- input DMAs, spread over queues in priority order --------
            src_t = []
            for v in range(V):
                t = consts.tile([H, C, W], bf16, tag=f"src{v}")
                src_t.append(t)
            ay_t = consts.tile([H, NY], bf16, tag="ay")
            # HW DGE queues (sync + scalar): the tensors the first matmuls
            # need, split across both queues
            nc.sync.dma_start(out=ay_t[:, 0:DLOC * H],
                              in_=ay.ap()[:, 0:DLOC * H])
            nc.scalar.dma_start(out=src_t[0][:, 0:8, :],
                                in_=srcT.ap()[0, :, 0:8, :])
            nc.sync.dma_start(out=src_t[0][:, 8:16, :],
                              in_=srcT.ap()[0, :, 8:16, :])
            nc.scalar.dma_start(out=ay_t[:, DLOC * H:],
                                in_=ay.ap()[:, DLOC * H:])
            # gpsimd SWDGE queue (Pool is otherwise idle early): the rest
            nc.gpsimd.dma_start(out=src_t[1], in_=srcT.ap()[1])
            nc.gpsimd.dma_start(out=src_t[2], in_=srcT.ap()[2])
            nc.gpsimd.dma_start(out=src_t[3], in_=srcT.ap()[3])
            bx_t = consts.tile([W, NX], bf16, tag="bx")
            nc.gpsimd.dma_start(out=bx_t, in_=bx.ap())
            cur_t = consts.tile([W, C, H], bf16, tag="cur")
            nc.gpsimd.dma_start(out=cur_t, in_=cur.ap())

            # ---- stage 1: y-interpolation --------------------------------
            # T'[w, c, (d,py)] = sum_h src[v][h,c,w] * Ay[h,(d,py)]
            # The PE costs ~427ns per matmul regardless (213ns stream +
            # ~214ns fixed per-instruction overhead), so the 2-lane
            # ACT/DVE cast pipe keeps up easily; ACT takes most.
            tps = [tp_pool.tile([W, C, DLOC * H], bf16, tag=f"tp{v}",
                                name=f"tp{v}") for v in range(V)]
            cast_pat = ["a", "v"]
            ci = 0
            for v in range(V):
                rhs = ay_t[:, v * DLOC * H:(v + 1) * DLOC * H]
                for cq in range(8):
                    c0 = cq * 2
                    ps1 = ps1_pool.tile([W, 2, DLOC * H], fp32, tag="ps1",
                                        name="ps1")
                    for cc in range(2):
                        nc.tensor.matmul(
                            ps1[:, cc, :], src_t[v][:, c0 + cc, :], rhs,
                            start=True, stop=True)
                    dst = tps[v][:, c0:c0 + 2, :]
                    if cast_pat[ci % 2] == "a":
                        nc.scalar.copy(dst, ps1)
                    else:
                        nc.vector.tensor_copy(dst, ps1)
                    ci += 1

            # ---- stage 2: x-interpolation + view-sum + channel dot -------
            # (single-matmul N=1024 fails the s3d3_mm_num_elements ISA
            # check: a matmul writes at most one 512-elem PSUM bank; the
            # two half-matmuls share their Bx weight load back-to-back)
            # Epilogue is software-pipelined: RED(d) is emitted after
            # TT0(d+1) to match readiness order on the in-order DVE.
            osb = osb_pool.tile([W, DLOC, H], fp32, tag="osb", name="osb")
            tmps = []

            def red(d):
                nc.vector.tensor_reduce(
                    osb[:, d, :], tmps[d].transpose([0, 2, 1]),
                    axis=Axis.X, op=Alu.add)
                if d == DLOC // 2 - 1:
                    nc.sync.dma_start(out=out.ap()[:, 0:DLOC // 2, :],
                                      in_=osb[:, 0:DLOC // 2, :])
                elif d == DLOC - 1:
                    nc.sync.dma_start(out=out.ap()[:, DLOC // 2:, :],
                                      in_=osb[:, DLOC // 2:, :])

            for d in range(DLOC):
                ps2 = ps2_pool.tile([W, C, H], fp32, tag="ps2", name="ps2")
                for v in range(V):
                    lhsT = bx_t[:, (v * DLOC + d) * W:(v * DLOC + d + 1) * W]
                    for half in range(2):
                        nc.tensor.matmul(
                            ps2[:, half * 8:half * 8 + 8, :], lhsT,
                            tps[v][:, half * 8:half * 8 + 8,
                                   d * H:(d + 1) * H],
                            start=(v == 0), stop=(v == V - 1))
                tmp_d = tmp_pool.tile([W, C, H], bf16, tag="tmp", name="tmp")
                tmps.append(tmp_d)
                nc.vector.tensor_mul(tmp_d[:, 0:8, :], ps2[:, 0:8, :],
                                     cur_t[:, 0:8, :])
                if d > 0:
                    red(d - 1)
                if d < DLOC - 1:
                    # Pool multiplies the second half (via an ACT cast,
                    # since Pool can't read PSUM)
                    s2 = s2_pool.tile([W, 8, H], bf16, tag="s2", name="s2")
                    nc.scalar.copy(s2, ps2[:, 8:16, :])
                    nc.gpsimd.tensor_mul(tmp_d[:, 8:16, :], s2,
                                         cur_t[:, 8:16, :])
                else:
                    # last plane: shortest chain, straight on DVE
                    nc.vector.tensor_mul(tmp_d[:, 8:16, :], ps2[:, 8:16, :],
                                         cur_t[:, 8:16, :])
            red(DLOC - 1)

    nc.compile()
    return nc


def _get_nc():
    if "nc" not in _CACHE:
        _CACHE["nc"] = _build_nc()
    return _CACHE["nc"]


# --------------------------------------------------------------------------
# Host-side geometry
# --------------------------------------------------------------------------
def _depth_planes(min_depth, max_depth):
    """Mimic the reference's fp32 arithmetic."""
    ramp = np.linspace(0.0, 1.0, D, dtype=np.float32)
    inv_min = (np.float32(1.0) / np.float32(min_depth)).astype(np.float32)
    inv_max = (np.float32(1.0) / np.float32(max_depth)).astype(np.float32)
    return (np.float32(1.0) /
            (inv_min + (inv_max - inv_min) * ramp).astype(np.float32))


def _is_separable(src_extrinsics, src_Ks, cur_invK):
    E = src_extrinsics[0]          # (V,4,4)
    K = src_Ks[0]                  # (V,4,4)
    iK = cur_invK[0]               # (4,4)
    eye3 = np.eye(3, dtype=E.dtype)
    for v in range(V):
        if not np.array_equal(E[v, :3, :3], eye3):
            return False
        if not np.array_equal(E[v, 3], np.array([0, 0, 0, 1], dtype=E.dtype)):
            return False
        k = K[v]
        if not (k[0, 1] == 0 and k[0, 3] == 0 and k[1, 0] == 0 and k[1, 3] == 0
                and np.array_equal(k[2], np.array([0, 0, 1, 0], dtype=K.dtype))):
            return False
    if not (iK[0, 1] == 0 and iK[1, 0] == 0 and iK[2, 0] == 0
            and iK[2, 1] == 0 and iK[2, 2] == 1):
        return False
    return True


def _coords(src_extrinsics, src_Ks, cur_invK, depths):
    """Per-(view, plane) 1-D sample coordinates: x[v,d,px], y[v,d,py]."""
    E = src_extrinsics[0].astype(np.float64)
    K = src_Ks[0].astype(np.float64)
    iK = cur_invK[0].astype(np.float64)
    i00, i02 = iK[0, 0], iK[0, 2]
    i11, i12 = iK[1, 1], iK[1, 2]
    px = np.arange(W, dtype=np.float64) + 0.5
    py = np.arange(H, dtype=np.float64) + 0.5
    xcs = np.empty((V, D, W), np.float64)
    ycs = np.empty((V, D, H), np.float64)
    for v in range(V):
        k00, k02 = K[v, 0, 0], K[v, 0, 2]
        k11, k12 = K[v, 1, 1], K[v, 1, 2]
        tx, ty, tz = E[v, 0, 3], E[v, 1, 3], E[v, 2, 3]
        for d in range(D):
            Dd = float(depths[d])
            z32 = np.float32(depths[d]) + np.float32(tz)        # ref fp32 z
            if not (z32 > 0):
                xcs[v, d] = OOB
                ycs[v, d] = OOB
                continue
            Zs = float(np.float32(z32 + np.float32(EPS)))
            rx = i00 * px + i02
            ry = i11 * py + i12
            u = (k00 * rx * Dd + k02 * Dd + k00 * tx + k02 * tz) / Zs
            vv = (k11 * ry * Dd + k12 * Dd + k11 * ty + k12 * tz) / Zs
            xcs[v, d] = np.clip(np.nan_to_num(u - 0.5, nan=OOB,
                                              posinf=OOB, neginf=-OOB),
                                -OOB, OOB)
            ycs[v, d] = np.clip(np.nan_to_num(vv - 0.5, nan=OOB,
                                              posinf=OOB, neginf=-OOB),
                                -OOB, OOB)
    return xcs.astype(np.float32), ycs.astype(np.float32)


# --------------------------------------------------------------------------
# Exact numpy fallback (general geometry)
# --------------------------------------------------------------------------
def _reference_numpy(cur_feats, src_feats, src_extrinsics, src_Ks, cur_invK,
                     min_depth, max_depth):
    f32 = np.float32
    N = H * W
    dp = _depth_planes(min_depth.reshape(-1)[0], max_depth.reshape(-1)[0])
    xx, yy = np.meshgrid(np.arange(W, dtype=f32) + 0.5,
                         np.arange(H, dtype=f32) + 0.5)
    pix = np.stack([xx.ravel(), yy.ravel(), np.ones(N, f32)], 0)       # (3,N)
    rays = cur_invK[0, :3, :3].astype(f32) @ pix                       # (3,N)
    world = rays[None] * dp[:, None, None]                             # (D,3,N)
    world4 = np.concatenate([world, np.ones((D, 1, N), f32)], 1)       # (D,4,N)
    P = np.einsum("vij,vjk->vik", src_Ks[0], src_extrinsics[0])[:, :3]  # (V,3,4)
    cam = np.einsum("vij,djn->vdin", P, world4).astype(f32)            # (V,D,3,N)
    z = cam[:, :, 2]
    u = cam[:, :, 0] / (z + f32(EPS))
    vv = cam[:, :, 1] / (z + f32(EPS))
    x = (u - 0.5).astype(f32).reshape(V, D * N)
    y = (vv - 0.5).astype(f32).reshape(V, D * N)
    out = np.zeros((D, H, W), f32)
    cur = cur_feats[0].reshape(C, N)                                   # (C,N)
    for v in range(V):
        f = src_feats[0, v].reshape(C, N)
        x0 = np.floor(x[v])
        y0 = np.floor(y[v])
        acc = np.zeros((C, D * N), f32)
        for dx in (0.0, 1.0):
            for dy in (0.0, 1.0):
                xi = x0 + dx
                yi = y0 + dy
                wgt = (1.0 - np.abs(x[v] - xi)) * (1.0 - np.abs(y[v] - yi))
                valid = ((xi >= 0) & (xi < W) & (yi >= 0) & (yi < H))
                idx = (np.clip(yi, 0, H - 1) * W +
                       np.clip(xi, 0, W - 1)).astype(np.int64)
                acc += f[:, idx] * (wgt * valid.astype(f32))[None]
        dot = (acc.reshape(C, D, N) *
               cur[:, None, :]).sum(0)                                 # (D,N)
        mask = (z[v] > 0).astype(f32)                                  # (D,N)
        out += (dot * mask).reshape(D, H, W)
    return out[None].astype(np.float32)


# --------------------------------------------------------------------------
# Entry points
# --------------------------------------------------------------------------
def _prepare_inputs(cur_feats, src_feats, src_extrinsics, src_Ks, cur_invK,
                    min_depth, max_depth):
    bf16 = ml_dtypes.bfloat16
    dp = _depth_planes(min_depth.reshape(-1)[0], max_depth.reshape(-1)[0])
    xcs, ycs = _coords(src_extrinsics, src_Ks, cur_invK, dp)  # (V,D,W),(V,D,H)
    # hat matrices (exact bilinear factorization), built on host
    hg = np.arange(H, dtype=np.float32)
    wg = np.arange(W, dtype=np.float32)
    Ay = np.maximum(np.float32(0.0),
                    np.float32(1.0) -
                    np.abs(ycs[None] - hg[:, None, None, None]))  # (H,V,D,H)
    Bx = np.maximum(np.float32(0.0),
                    np.float32(1.0) -
                    np.abs(xcs[None] - wg[:, None, None, None]))  # (W,V,D,W)
    srcT = np.ascontiguousarray(
        src_feats[0].transpose(0, 2, 1, 3)).astype(bf16)          # (V,H,C,W)
    curT = np.ascontiguousarray(
        cur_feats[0].transpose(2, 0, 1)).astype(bf16)             # (W,C,H)
    in_maps = []
    for k in range(N_CORES):
        sl = slice(k * DLOC, (k + 1) * DLOC)
        in_maps.append({
            "srcT": srcT,
            "ay": np.ascontiguousarray(Ay[:, :, sl, :]).reshape(H, NY).astype(bf16),
            "bx": np.ascontiguousarray(Bx[:, :, sl, :]).reshape(W, NX).astype(bf16),
            "cur": curT,
        })
    return in_maps


def _run(inputs, trace=False):
    from concourse.bass_utils import run_bass_kernel_spmd
    nc = _get_nc()
    in_maps = _prepare_inputs(**inputs)
    res = run_bass_kernel_spmd(nc, in_maps, core_ids=list(range(N_CORES)),
                               trace=trace)
    # device out is [W, DLOC, H] -> (DLOC, H, W)
    parts = [res.results[k]["out"].transpose(1, 2, 0) for k in range(N_CORES)]
    out = np.concatenate(parts, 0)[None].astype(np.float32)
    return out, res


def kernel(cur_feats, src_feats, src_extrinsics, src_Ks, cur_invK,
           min_depth, max_depth):
    args = dict(cur_feats=np.asarray(cur_feats), src_feats=np.asarray(src_feats),
                src_extrinsics=np.asarray(src_extrinsics),
                src_Ks=np.asarray(src_Ks), cur_invK=np.asarray(cur_invK),
                min_depth=np.asarray(min_depth), max_depth=np.asarray(max_depth))
    if not _is_separable(args["src_extrinsics"], args["src_Ks"],
                         args["cur_invK"]):
        return _reference_numpy(**args)
    out, _ = _run(args)
    return out


# revision 23
# speedup vs baseline: 1.1471x; 1.1471x over previous
"""Plane-sweep cost-volume kernel for Trainium2 (8 NeuronCores).

Problem shape (hardcoded): B=1, V=4 source views, C=16 feature channels,
H=64, W=96, D=64 depth planes.  Output: (1, D, H, W) float32.

Strategy
--------
The benchmark geometry has identity rotations (extrinsics are pure
translations) and zero-skew pinhole intrinsics, so for each (view, depth
plane) the warp from output pixels to source-image sample coordinates is an
axis-separable affine map:  x = ax + bx*px,  y = ay + by*py.  Bilinear
grid_sample with zero padding then factorizes exactly into two 1-D linear
interpolations, each a small dense matrix of "hat" functions
hat(t - k) = max(0, 1 - |t - k|):

    warped_c = Ay(v,d) @ src_c @ Bx(v,d)^T        (exactly equal to
                                                   grid_sample zeros/bilinear)

so the whole cost volume becomes TensorEngine matmuls — no gathers.  The
view sum  sum_v  is accumulated in PSUM, and the channel dot with cur_feats
is a vector multiply + strided reduce.

Sharding: depth planes across the 8 cores (8 planes each); features are
replicated.  The hat matrices Ay/Bx are built on host directly in bf16 and
DMA'd (the previous version built them on device from broadcast DMAs, which
serialized ~25us of startup in front of the first matmul).  DMAs are spread
across the HW-DGE queues (sync/scalar) plus vector/gpsimd SWDGE queues so
the tensor engine starts within ~2us.  PSUM->SBUF casts round-robin over
the DVE/Pool/Act engines so the PE is never cast-throttled (which would
also hold it at the 1.2GHz mid p-state).

If the inputs do not have the separable structure (rotations != identity or
non-pinhole intrinsics), we fall back to an exact numpy implementation.
"""

import numpy as np
import ml_dtypes

H, W, D, V, C = 64, 96, 64, 4, 16
N_CORES = 8
DLOC = D // N_CORES            # 8 depth planes per core
NY = V * DLOC * H              # 2048
NX = V * DLOC * W              # 3072
EPS = 1e-8
OOB = 1.0e9                    # sample coord pushed out of range => zero weights

_CACHE = {}


# --------------------------------------------------------------------------
# Device kernel
# --------------------------------------------------------------------------
def _build_nc():
    import concourse.bacc as bacc
    import concourse.tile as tile
    from concourse import mybir

    fp32 = mybir.dt.float32
    bf16 = mybir.dt.bfloat16
    Alu = mybir.AluOpType
    Axis = mybir.AxisListType

    nc = bacc.Bacc("TRN2", target_bir_lowering=False, debug=False,
                   num_devices=N_CORES)

    srcT = nc.dram_tensor("srcT", [V, H, C, W], bf16, kind="ExternalInput")
    ay = nc.dram_tensor("ay", [H, NY], bf16, kind="ExternalInput")
    bx = nc.dram_tensor("bx", [W, NX], bf16, kind="ExternalInput")
    cur = nc.dram_tensor("cur", [W, C, H], bf16, kind="ExternalInput")
    out = nc.dram_tensor("out", [W, DLOC, H], fp32, kind="ExternalOutput")

    with tile.TileContext(nc) as tc:
        with (
            tc.tile_pool(name="consts", bufs=1) as consts,
            tc.tile_pool(name="tp", bufs=1) as tp_pool,
            tc.tile_pool(name="ps1", bufs=4, space="PSUM") as ps1_pool,
            tc.tile_pool(name="ps2", bufs=2, space="PSUM") as ps2_pool,
            tc.tile_pool(name="s2", bufs=2) as s2_pool,
            tc.tile_pool(name="tmp", bufs=3) as tmp_pool,
            tc.tile_pool(name="osb", bufs=1) as osb_pool,
        ):
            # ---- input DMAs, spread over queues in priority order --------
            src_t = []
            for v in range(V):
                t = consts.tile([H, C, W], bf16, tag=f"src{v}")
                src_t.append(t)
            ay_t = consts.tile([H, NY], bf16, tag="ay")
            # HW DGE queues (sync + scalar): the tensors the first matmuls
            # need, split across both queues
            nc.sync.dma_start(out=ay_t[:, 0:DLOC * H],
                              in_=ay.ap()[:, 0:DLOC * H])
            nc.scalar.dma_start(out=src_t[0][:, 0:8, :],
                                in_=srcT.ap()[0, :, 0:8, :])
            nc.sync.dma_start(out=src_t[0][:, 8:16, :],
                              in_=srcT.ap()[0, :, 8:16, :])
            nc.scalar.dma_start(out=ay_t[:, DLOC * H:],
                                in_=ay.ap()[:, DLOC * H:])
            # gpsimd SWDGE queue (Pool is otherwise idle early): the rest
            nc.gpsimd.dma_start(out=src_t[1], in_=srcT.ap()[1])
            nc.gpsimd.dma_start(out=src_t[2], in_=srcT.ap()[2])
            nc.gpsimd.dma_start(out=src_t[3], in_=srcT.ap()[3])
            bx_t = consts.tile([W, NX], bf16, tag="bx")
            nc.gpsimd.dma_start(out=bx_t, in_=bx.ap())
            cur_t = consts.tile([W, C, H], bf16, tag="cur")
            nc.gpsimd.dma_start(out=cur_t, in_=cur.ap())

            # ---- stage 1: y-interpolation --------------------------------
            # T'[w, c, (d,py)] = sum_h src[v][h,c,w] * Ay[h,(d,py)]
            # The PE costs ~427ns per matmul regardless (213ns stream +
            # ~214ns fixed per-instruction overhead), so the 2-lane
            # ACT/DVE cast pipe keeps up easily; ACT takes most.
            tps = [tp_pool.tile([W, C, DLOC * H], bf16, tag=f"tp{v}",
                                name=f"tp{v}") for v in range(V)]
            cast_pat = ["a", "v"]
            ci = 0
            for v in range(V):
                rhs = ay_t[:, v * DLOC * H:(v + 1) * DLOC * H]
                for c in range(C):
                    ps1 = ps1_pool.tile([W, DLOC * H], fp32, tag="ps1",
                                        name="ps1")
                    nc.tensor.matmul(
                        ps1, src_t[v][:, c, :], rhs,
                        start=True, stop=True)
                    dst = tps[v][:, c, :]
                    if cast_pat[ci % 2] == "a":
                        nc.scalar.copy(dst, ps1)
                    else:
                        nc.vector.tensor_copy(dst, ps1)
                    ci += 1

            # ---- stage 2: x-interpolation + view-sum + channel dot -------
            # (single-matmul N=1024 fails the s3d3_mm_num_elements ISA
            # check: a matmul writes at most one 512-elem PSUM bank; the
            # two half-matmuls share their Bx weight load back-to-back)
            # Epilogue is software-pipelined: RED(d) is emitted after
            # TT0(d+1) to match readiness order on the in-order DVE.
            osb = osb_pool.tile([W, DLOC, H], fp32, tag="osb", name="osb")
            tmps = []

            def red(d):
                nc.vector.tensor_reduce(
                    osb[:, d, :], tmps[d].transpose([0, 2, 1]),
                    axis=Axis.X, op=Alu.add)
                if d == DLOC // 2 - 1:
                    nc.sync.dma_start(out=out.ap()[:, 0:DLOC // 2, :],
                                      in_=osb[:, 0:DLOC // 2, :])
                elif d == DLOC - 1:
                    nc.sync.dma_start(out=out.ap()[:, DLOC // 2:, :],
                                      in_=osb[:, DLOC // 2:, :])

            for d in range(DLOC):
                ps2a = ps2_pool.tile([W, 8, H], fp32, tag="ps2", name="ps2a")
                ps2b = ps2_pool.tile([W, 8, H], fp32, tag="ps2b", name="ps2b")
                for v in range(V):
                    lhsT = bx_t[:, (v * DLOC + d) * W:(v * DLOC + d + 1) * W]
                    for half, pt in ((0, ps2a), (1, ps2b)):
                        nc.tensor.matmul(
                            pt, lhsT,
                            tps[v][:, half * 8:half * 8 + 8,
                                   d * H:(d + 1) * H],
                            start=(v == 0), stop=(v == V - 1))
                tmp_d = tmp_pool.tile([W, C, H], bf16, tag="tmp", name="tmp")
                tmps.append(tmp_d)
                nc.vector.tensor_mul(tmp_d[:, 0:8, :], ps2a,
                                     cur_t[:, 0:8, :])
                if d > 0:
                    red(d - 1)
                if d < DLOC - 1:
                    # Pool multiplies the second half (via an ACT cast,
                    # since Pool can't read PSUM)
                    s2 = s2_pool.tile([W, 8, H], bf16, tag="s2", name="s2")
                    nc.scalar.copy(s2, ps2b)
                    nc.gpsimd.tensor_mul(tmp_d[:, 8:16, :], s2,
                                         cur_t[:, 8:16, :])
                else:
                    # last plane: shortest chain, straight on DVE
                    nc.vector.tensor_mul(tmp_d[:, 8:16, :], ps2b,
                                         cur_t[:, 8:16, :])
            red(DLOC - 1)

    nc.compile()
    return nc


def _get_nc():
    if "nc" not in _CACHE:
        _CACHE["nc"] = _build_nc()
    return _CACHE["nc"]


# --------------------------------------------------------------------------
# Host-side geometry
# --------------------------------------------------------------------------
def _depth_planes(min_depth, max_depth):
    """Mimic the reference's fp32 arithmetic."""
    ramp = np.linspace(0.0, 1.0, D, dtype=np.float32)
    inv_min = (np.float32(1.0) / np.float32(min_depth)).astype(np.float32)
    inv_max = (np.float32(1.0) / np.float32(max_depth)).astype(np.float32)
    return (np.float32(1.0) /
            (inv_min + (inv_max - inv_min) * ramp).astype(np.float32))


def _is_separable(src_extrinsics, src_Ks, cur_invK):
    E = src_extrinsics[0]          # (V,4,4)
    K = src_Ks[0]                  # (V,4,4)
    iK = cur_invK[0]               # (4,4)
    eye3 = np.eye(3, dtype=E.dtype)
    for v in range(V):
        if not np.array_equal(E[v, :3, :3], eye3):
            return False
        if not np.array_equal(E[v, 3], np.array([0, 0, 0, 1], dtype=E.dtype)):
            return False
        k = K[v]
        if not (k[0, 1] == 0 and k[0, 3] == 0 and k[1, 0] == 0 and k[1, 3] == 0
                and np.array_equal(k[2], np.array([0, 0, 1, 0], dtype=K.dtype))):
            return False
    if not (iK[0, 1] == 0 and iK[1, 0] == 0 and iK[2, 0] == 0
            and iK[2, 1] == 0 and iK[2, 2] == 1):
        return False
    return True


def _coords(src_extrinsics, src_Ks, cur_invK, depths):
    """Per-(view, plane) 1-D sample coordinates: x[v,d,px], y[v,d,py]."""
    E = src_extrinsics[0].astype(np.float64)
    K = src_Ks[0].astype(np.float64)
    iK = cur_invK[0].astype(np.float64)
    i00, i02 = iK[0, 0], iK[0, 2]
    i11, i12 = iK[1, 1], iK[1, 2]
    px = np.arange(W, dtype=np.float64) + 0.5
    py = np.arange(H, dtype=np.float64) + 0.5
    xcs = np.empty((V, D, W), np.float64)
    ycs = np.empty((V, D, H), np.float64)
    for v in range(V):
        k00, k02 = K[v, 0, 0], K[v, 0, 2]
        k11, k12 = K[v, 1, 1], K[v, 1, 2]
        tx, ty, tz = E[v, 0, 3], E[v, 1, 3], E[v, 2, 3]
        for d in range(D):
            Dd = float(depths[d])
            z32 = np.float32(depths[d]) + np.float32(tz)        # ref fp32 z
            if not (z32 > 0):
                xcs[v, d] = OOB
                ycs[v, d] = OOB
                continue
            Zs = float(np.float32(z32 + np.float32(EPS)))
            rx = i00 * px + i02
            ry = i11 * py + i12
            u = (k00 * rx * Dd + k02 * Dd + k00 * tx + k02 * tz) / Zs
            vv = (k11 * ry * Dd + k12 * Dd + k11 * ty + k12 * tz) / Zs
            xcs[v, d] = np.clip(np.nan_to_num(u - 0.5, nan=OOB,
                                              posinf=OOB, neginf=-OOB),
                                -OOB, OOB)
            ycs[v, d] = np.clip(np.nan_to_num(vv - 0.5, nan=OOB,
                                              posinf=OOB, neginf=-OOB),
                                -OOB, OOB)
    return xcs.astype(np.float32), ycs.astype(np.float32)


# --------------------------------------------------------------------------
# Exact numpy fallback (general geometry)
# --------------------------------------------------------------------------
def _reference_numpy(cur_feats, src_feats, src_extrinsics, src_Ks, cur_invK,
                     min_depth, max_depth):
    f32 = np.float32
    N = H * W
    dp = _depth_planes(min_depth.reshape(-1)[0], max_depth.reshape(-1)[0])
    xx, yy = np.meshgrid(np.arange(W, dtype=f32) + 0.5,
                         np.arange(H, dtype=f32) + 0.5)
    pix = np.stack([xx.ravel(), yy.ravel(), np.ones(N, f32)], 0)       # (3,N)
    rays = cur_invK[0, :3, :3].astype(f32) @ pix                       # (3,N)
    world = rays[None] * dp[:, None, None]                             # (D,3,N)
    world4 = np.concatenate([world, np.ones((D, 1, N), f32)], 1)       # (D,4,N)
    P = np.einsum("vij,vjk->vik", src_Ks[0], src_extrinsics[0])[:, :3]  # (V,3,4)
    cam = np.einsum("vij,djn->vdin", P, world4).astype(f32)            # (V,D,3,N)
    z = cam[:, :, 2]
    u = cam[:, :, 0] / (z + f32(EPS))
    vv = cam[:, :, 1] / (z + f32(EPS))
    x = (u - 0.5).astype(f32).reshape(V, D * N)
    y = (vv - 0.5).astype(f32).reshape(V, D * N)
    out = np.zeros((D, H, W), f32)
    cur = cur_feats[0].reshape(C, N)                                   # (C,N)
    for v in range(V):
        f = src_feats[0, v].reshape(C, N)
        x0 = np.floor(x[v])
        y0 = np.floor(y[v])
        acc = np.zeros((C, D * N), f32)
        for dx in (0.0, 1.0):
            for dy in (0.0, 1.0):
                xi = x0 + dx
                yi = y0 + dy
                wgt = (1.0 - np.abs(x[v] - xi)) * (1.0 - np.abs(y[v] - yi))
                valid = ((xi >= 0) & (xi < W) & (yi >= 0) & (yi < H))
                idx = (np.clip(yi, 0, H - 1) * W +
                       np.clip(xi, 0, W - 1)).astype(np.int64)
                acc += f[:, idx] * (wgt * valid.astype(f32))[None]
        dot = (acc.reshape(C, D, N) *
               cur[:, None, :]).sum(0)                                 # (D,N)
        mask = (z[v] > 0).astype(f32)                                  # (D,N)
        out += (dot * mask).reshape(D, H, W)
    return out[None].astype(np.float32)


# --------------------------------------------------------------------------
# Entry points
# --------------------------------------------------------------------------
def _prepare_inputs(cur_feats, src_feats, src_extrinsics, src_Ks, cur_invK,
                    min_depth, max_depth):
    bf16 = ml_dtypes.bfloat16
    dp = _depth_planes(min_depth.reshape(-1)[0], max_depth.reshape(-1)[0])
    xcs, ycs = _coords(src_extrinsics, src_Ks, cur_invK, dp)  # (V,D,W),(V,D,H)
    # hat matrices (exact bilinear factorization), built on host
    hg = np.arange(H, dtype=np.float32)
    wg = np.arange(W, dtype=np.float32)
    Ay = np.maximum(np.float32(0.0),
                    np.float32(1.0) -
                    np.abs(ycs[None] - hg[:, None, None, None]))  # (H,V,D,H)
    Bx = np.maximum(np.float32(0.0),
                    np.float32(1.0) -
                    np.abs(xcs[None] - wg[:, None, None, None]))  # (W,V,D,W)
    srcT = np.ascontiguousarray(
        src_feats[0].transpose(0, 2, 1, 3)).astype(bf16)          # (V,H,C,W)
    curT = np.ascontiguousarray(
        cur_feats[0].transpose(2, 0, 1)).astype(bf16)             # (W,C,H)
    in_maps = []
    for k in range(N_CORES):
        sl = slice(k * DLOC, (k + 1) * DLOC)
        in_maps.append({
            "srcT": srcT,
            "ay": np.ascontiguousarray(Ay[:, :, sl, :]).reshape(H, NY).astype(bf16),
            "bx": np.ascontiguousarray(Bx[:, :, sl, :]).reshape(W, NX).astype(bf16),
            "cur": curT,
        })
    return in_maps


def _run(inputs, trace=False):
    from concourse.bass_utils import run_bass_kernel_spmd
    nc = _get_nc()
    in_maps = _prepare_inputs(**inputs)
    res = run_bass_kernel_spmd(nc, in_maps, core_ids=list(range(N_CORES)),
                               trace=trace)
    # device out is [W, DLOC, H] -> (DLOC, H, W)
    parts = [res.results[k]["out"].transpose(1, 2, 0) for k in range(N_CORES)]
    out = np.concatenate(parts, 0)[None].astype(np.float32)
    return out, res


def kernel(cur_feats, src_feats, src_extrinsics, src_Ks, cur_invK,
           min_depth, max_depth):
    args = dict(cur_feats=np.asarray(cur_feats), src_feats=np.asarray(src_feats),
                src_extrinsics=np.asarray(src_extrinsics),
                src_Ks=np.asarray(src_Ks), cur_invK=np.asarray(cur_invK),
                min_depth=np.asarray(min_depth), max_depth=np.asarray(max_depth))
    if not _is_separable(args["src_extrinsics"], args["src_Ks"],
                         args["cur_invK"]):
        return _reference_numpy(**args)
    out, _ = _run(args)
    return out
